# revision 1
# baseline (speedup 1.0000x reference)
"""Trainium2 Bass kernel for nn_DenseExpert (soft-gated mixture of dense experts).

Math:  out[b,u] = sum_e gate[b,e] * (x[b,:] @ alpha[e]) [u] + (gate @ beta)[b,u]

Strategy (pure data parallel over batch, 8 cores). Per 512-row chunk per core:
  1. DMA x/gate chunk (batch-major); cast to fp16 (11-bit mantissa, close to
     TF32 precision; PE streams fp16 at 1 cycle/row; PSUM accumulation fp32).
  2. Build K=64 block-diagonal gate matrices: dstack[p, e, c] =
     gate[p,e]*[c == p%64], one fp16 tensor_tensor per 128-row tile on DVE.
     The gate operand is a host-duplicated pair layout (j-dim) so every
     operand ends with stride-1 fp16 -> DVE 2x_1p dual-pump mode.
  3. y_e.T tiles via PE matmuls: for each 64-row block l,
     yT[i, (e, c)] = x[64l:64l+64, :].T @ dstack[64l:64l+64]   (N=512).
     This replaces both a scale stage and per-expert PE transposes.
  4. PSUM->SBUF copies gather yT into [i, e, b] fp16 layout (DVE/ACT split).
  5. PE matmuls accumulate out.T[u,b] = sum_e alpha_e.T @ y_e.T.  The bias
     gate @ beta (1.5% of FLOPs) is added on the host during assembly.
  6. out.T (fp32) copied to SBUF and DMA'd to DRAM in [U, B] layout; the
     host does the final cheap transpose when assembling the full result.
"""

import dataclasses
from contextlib import ExitStack

import numpy as np

import concourse.bacc as bacc
import concourse.tile as tile
import concourse.mybir as mybir
from concourse.bass_utils import run_bass_kernel_spmd

F32 = mybir.dt.float32
F16 = mybir.dt.float16

B, E, I, U = 65536, 8, 128, 128
NCORES = 8
BLOC = B // NCORES        # 8192 batch rows per core
CHUNK = 512               # batch rows per pipeline chunk
NCHUNK = BLOC // CHUNK    # 16
TPC = CHUNK // 128        # 128-row tiles per chunk
KB = 64                   # contraction block for the diag trick


def _build():
    nc = bacc.Bacc("TRN2", target_bir_lowering=False, debug=False)

    x = nc.dram_tensor("x", [BLOC, I], F32, kind="ExternalInput").ap()
    gate = nc.dram_tensor("gate", [BLOC, E], F32, kind="ExternalInput").ap()
    # g2[q, c, t, e, j] = gate[512c + 128t + q, e] duplicated over j in {0,1}
    g2 = nc.dram_tensor("g2", [128, NCHUNK, TPC, E, 2], F16, kind="ExternalInput").ap()
    alpha = nc.dram_tensor("alpha", [E, I, U], F32, kind="ExternalInput").ap()
    beta = nc.dram_tensor("beta", [E, U], F32, kind="ExternalInput").ap()
    ident = nc.dram_tensor("ident", [128, 128], F16, kind="ExternalInput").ap()
    idrep = nc.dram_tensor("idrep", [128, E, KB], F16, kind="ExternalInput").ap()
    # output stays feature-major on HW; host transposes when assembling
    outT = nc.dram_tensor("outT", [U, BLOC], F32, kind="ExternalOutput").ap()

    with tile.TileContext(nc) as tc, ExitStack() as ctx:
        const = ctx.enter_context(tc.tile_pool(name="const", bufs=1))
        xp = ctx.enter_context(tc.tile_pool(name="xp", bufs=6))
        dgp = ctx.enter_context(tc.tile_pool(name="dgp", bufs=8))
        ytp = ctx.enter_context(tc.tile_pool(name="ytp", bufs=4))
        op = ctx.enter_context(tc.tile_pool(name="op", bufs=3))
        ps_yt = ctx.enter_context(tc.tile_pool(name="ps_yt", bufs=3, space="PSUM"))
        ps_ot = ctx.enter_context(tc.tile_pool(name="ps_ot", bufs=1, space="PSUM"))

        # --- constants (cast alpha/beta to fp16 on chip) ---
        alpha_sb = const.tile([128, E, U], F32, tag="alpha")
        nc.sync.dma_start(alpha_sb[:], alpha.rearrange("e i u -> i e u"))
        alpha_h = const.tile([128, E, U], F16, tag="alphah")
        nc.vector.tensor_copy(alpha_h[:], alpha_sb[:])

        g2_sb = const.tile([128, NCHUNK, TPC, E, 2], F16, tag="g2")
        nc.sync.dma_start(g2_sb[:], g2)
        g2_pitch = NCHUNK * TPC * E * 2
        ident_h = const.tile([128, 128], F16, tag="identh")
        nc.sync.dma_start(ident_h[:], ident)
        idrep_h = const.tile([128, E, KB], F16, tag="idreph")
        nc.sync.dma_start(idrep_h[:], idrep)

        def emit_front(c):
            row0 = c * CHUNK
            # x: SWDGE DMA with fused fp32->fp16 cast (issued from GpSimd,
            # which is otherwise idle)
            x_h = xp.tile([128, TPC, I], F16, tag="xh")
            nc.gpsimd.dma_start(
                x_h[:], x[row0 : row0 + CHUNK, :].rearrange("(t p) i -> p t i", p=128)
            )

            # per 128-row tile: diag build (DVE) + yT matmuls + gather copy
            yT_all = ytp.tile([128, E, TPC, 128], F16, tag="yT")
            for t in range(TPC):
                diag = dgp.tile([128, E, KB], F16, tag="diag")
                diag_v = dataclasses.replace(
                    diag[:],
                    ap=[[E * KB, 128], [KB, E], [2, KB // 2], [1, 2]],
                    offset=0,
                )
                idrep_v = dataclasses.replace(
                    idrep_h[:],
                    ap=[[E * KB, 128], [KB, E], [2, KB // 2], [1, 2]],
                    offset=0,
                )
                g2_v = dataclasses.replace(
                    g2_sb[:],
                    ap=[[g2_pitch, 128], [2, E], [0, KB // 2], [1, 2]],
                    offset=(c * TPC + t) * E * 2,
                )
                nc.vector.tensor_tensor(
                    diag_v, idrep_v, g2_v, op=mybir.AluOpType.mult
                )
                yT_ps = ps_yt.tile([128, 2, E, KB], F32, tag="yTps")
                for l in range(2):
                    nc.tensor.matmul(
                        yT_ps[:, l, :, :],
                        x_h[l * KB : (l + 1) * KB, t, :],
                        diag[l * KB : (l + 1) * KB, :, :],
                        start=True,
                        stop=True,
                    )
                dst = dataclasses.replace(
                    yT_all[:],
                    ap=[[E * TPC * 128, 128], [KB, 2], [TPC * 128, E], [1, KB]],
                    offset=t * 128,
                )
                if t == 3:
                    # last tile's gather entirely on DVE (ACT is the busier
                    # engine now that the diag builds run at 2x)
                    nc.vector.tensor_copy(dst, yT_ps[:])
                else:
                    nc.scalar.copy(dst, yT_ps[:])
            return yT_all

        def emit_back(c, yT_all):
            row0 = c * CHUNK
            oT_ps = ps_ot.tile([128, CHUNK], F32, tag="oTps")
            for e in range(E):
                nc.tensor.matmul(
                    oT_ps[:],
                    alpha_h[:, e, :],
                    yT_all[:, e, :, :],
                    start=(e == 0),
                    stop=(e == E - 1),
                )

            oT_sb = op.tile([128, CHUNK], F32, tag="oT")
            nc.vector.tensor_copy(oT_sb[:, : CHUNK // 2], oT_ps[:, : CHUNK // 2])
            nc.scalar.copy(oT_sb[:, CHUNK // 2 :], oT_ps[:, CHUNK // 2 :])
            nc.sync.dma_start(outT[:, row0 : row0 + CHUNK], oT_sb[:])

        pending = None
        for c in range(NCHUNK):
            front = emit_front(c)
            if pending is not None:
                emit_back(c - 1, pending)
            pending = front
        emit_back(NCHUNK - 1, pending)

    nc.compile()
    return nc


_NC_CACHE = None


def _make_idrep():
    idrep = np.zeros((128, E, KB), np.float16)
    for p in range(128):
        idrep[p, :, p % KB] = 1.0
    return idrep


def make_in_maps(x, gate_perc, alpha, beta):
    x = np.ascontiguousarray(np.asarray(x, dtype=np.float32))
    gate_perc = np.ascontiguousarray(np.asarray(gate_perc, dtype=np.float32))
    alpha = np.ascontiguousarray(np.asarray(alpha, dtype=np.float32))
    beta = np.ascontiguousarray(np.asarray(beta, dtype=np.float32))
    ident = np.eye(128, dtype=np.float16)
    idrep = _make_idrep()
    in_maps = []
    for c in range(NCORES):
        sl = slice(c * BLOC, (c + 1) * BLOC)
        # g2[q, ch, t, e, j] = gate[c*BLOC + 512ch + 128t + q, e], j duplicated
        gc = gate_perc[sl].astype(np.float16).reshape(NCHUNK, TPC, 128, E)
        gc = gc.transpose(2, 0, 1, 3)  # [q, ch, t, e]
        g2 = np.ascontiguousarray(
            np.broadcast_to(gc[..., None], (128, NCHUNK, TPC, E, 2))
        )
        in_maps.append(
            {
                "x": x[sl],
                "gate": gate_perc[sl],
                "alpha": alpha,
                "beta": beta,
                "ident": ident,
                "idrep": idrep,
                "g2": g2,
            }
        )
    return in_maps


def assemble(results, gate_perc, beta):
    # per-core outputs are [U, BLOC] f32; bias gate@beta added on host
    full_T = np.concatenate([results[c]["outT"] for c in range(NCORES)], axis=1)
    out = np.ascontiguousarray(full_T.T)
    out += np.asarray(gate_perc, dtype=np.float32) @ np.asarray(beta, dtype=np.float32)
    return out


def kernel(x, gate_perc, alpha, beta):
    global _NC_CACHE
    if _NC_CACHE is None:
        _NC_CACHE = _build()
    nc = _NC_CACHE

    in_maps = make_in_maps(x, gate_perc, alpha, beta)
    res = run_bass_kernel_spmd(nc, in_maps, list(range(NCORES))).results
    return assemble(res, gate_perc, beta)


if __name__ == "__main__":
    rng = np.random.default_rng(0)
    x = rng.standard_normal((B, I)).astype(np.float32)
    g = rng.random((B, E)).astype(np.float32)
    g /= g.sum(-1, keepdims=True)
    al = (rng.standard_normal((E, I, U)) * 0.05).astype(np.float32)
    be = (rng.standard_normal((E, U)) * 0.05).astype(np.float32)
    got = kernel(x, g, al, be)
    ref = np.einsum("bi,eio->beo", x, al, optimize=True)
    ref = np.einsum("beo,be->bo", ref, g) + g @ be
    err = np.abs(got - ref)
    print("max abs err", err.max(), "rel", err.max() / np.abs(ref).max())



# revision 8
# speedup vs baseline: 1.0865x; 1.0865x over previous
"""Trainium2 Bass kernel for nn_DenseExpert (soft-gated mixture of dense experts).

Math:  out[b,u] = sum_e gate[b,e] * (x[b,:] @ alpha[e]) [u] + (gate @ beta)[b,u]

Strategy (pure data parallel over batch, 8 cores). Per 512-row chunk per core:
  1. x is cast to fp16 and laid out [128, chunk, tile, I] on the host, so the
     per-chunk load is one contiguous HWDGE DMA (no gpsimd SWDGE cast).
  2. Per chunk, the K=64 block-diagonal gate matrices
     diag[p, t, e, c] = gate[512c+128t+p, e] * [c == p%64] are built by two
     tensor_tensor ops: tiles 0-1 on DVE (2x mode via host-duplicated gate
     pairs), tiles 2-3 on GPSIMD (all-SBUF, so it is allowed there).
  3. Front PE matmuls (gated transpose): for tile t and 64-row block l,
     yT[i, (e, c)] = x_block.T @ diag_block  (N=512), 2 blocks into one
     2-bank PSUM tile per 128-row tile.
  4. PSUM->SBUF gathers assemble yT_all[i, e, b] fp16; 2 gathers on ACT,
     2 on DVE per chunk (GPSIMD cannot touch PSUM) so neither engine
     exceeds the PE's per-chunk work.
  5. Back PE matmuls accumulate out.T[u,b] = sum_e alpha_e.T @ y_e.T into a
     double-buffered 1-bank PSUM tile; ACT copies it to SBUF as fp16.
  6. out.T fp16 DMA'd to DRAM [U, B]; host transposes/upcasts and adds the
     tiny bias gate @ beta while assembling the full result.
"""

import dataclasses
from contextlib import ExitStack

import numpy as np

import concourse.bacc as bacc
import concourse.tile as tile
import concourse.mybir as mybir
from concourse.bass_utils import run_bass_kernel_spmd

F32 = mybir.dt.float32
F16 = mybir.dt.float16

B, E, I, U = 65536, 8, 128, 128
NCORES = 8
BLOC = B // NCORES        # 8192 batch rows per core
CHUNK = 512               # batch rows per pipeline chunk
NCHUNK = BLOC // CHUNK    # 16
TPC = CHUNK // 128        # 128-row tiles per chunk
KB = 64                   # contraction block for the diag trick
NB = 128 // KB            # diag blocks per 128-row tile


def _build():
    nc = bacc.Bacc("TRN2", target_bir_lowering=False, debug=False)

    # xh[q, c, t, i] = x[512c + 128t + q, i] as fp16 (host-prepared)
    xh = nc.dram_tensor("xh", [128, NCHUNK, TPC, I], F16, kind="ExternalInput").ap()
    # g2[q, c, t, e, j] = gate[512c + 128t + q, e] duplicated over j in {0,1}
    g2 = nc.dram_tensor("g2", [128, NCHUNK, TPC, E, 2], F16, kind="ExternalInput").ap()
    alpha = nc.dram_tensor("alpha", [E, I, U], F32, kind="ExternalInput").ap()
    idrep = nc.dram_tensor("idrep", [128, E, KB], F16, kind="ExternalInput").ap()
    # output stays feature-major fp16 on HW; host transposes/upcasts
    outT = nc.dram_tensor("outT", [U, BLOC], F16, kind="ExternalOutput").ap()

    with tile.TileContext(nc) as tc, ExitStack() as ctx:
        const = ctx.enter_context(tc.tile_pool(name="const", bufs=1))
        xp = ctx.enter_context(tc.tile_pool(name="xp", bufs=4))
        dgp = ctx.enter_context(tc.tile_pool(name="dgp", bufs=3))
        ytp = ctx.enter_context(tc.tile_pool(name="ytp", bufs=3))
        op = ctx.enter_context(tc.tile_pool(name="op", bufs=3))
        # 8 PSUM banks: 3 x 2-bank yT tiles + 2 x 1-bank output buffers
        ps_yt = ctx.enter_context(tc.tile_pool(name="ps_yt", bufs=3, space="PSUM"))
        ps_ot = ctx.enter_context(tc.tile_pool(name="ps_ot", bufs=2, space="PSUM"))

        # --- constants (cast alpha to fp16 on chip) ---
        alpha_sb = const.tile([128, E, U], F32, tag="alpha")
        nc.sync.dma_start(alpha_sb[:], alpha.rearrange("e i u -> i e u"))
        alpha_h = const.tile([128, E, U], F16, tag="alphah")
        nc.vector.tensor_copy(alpha_h[:], alpha_sb[:])

        g2_sb = const.tile([128, NCHUNK, TPC, E, 2], F16, tag="g2")
        nc.sync.dma_start(g2_sb[:], g2)
        g2_pitch = NCHUNK * TPC * E * 2
        idrep_h = const.tile([128, E, KB], F16, tag="idreph")
        nc.sync.dma_start(idrep_h[:], idrep)

        # gather engine rotation per chunk (GPSIMD cannot access PSUM)
        gather_engines = [
            nc.scalar.copy,
            nc.vector.tensor_copy,
            nc.scalar.copy,
            nc.vector.tensor_copy,
        ]

        def emit_front(c):
            x_h = xp.tile([128, TPC, I], F16, tag="xh")
            nc.sync.dma_start(x_h[:], xh[:, c])

            # diag matrices: tiles 0-1 on DVE (2x stride-1 pairs), 2-3 on GPSIMD
            diag = dgp.tile([128, TPC, E, KB], F16, tag="diag")
            for eng, t0 in ((nc.vector, 0), (nc.gpsimd, 2)):
                diag_v = dataclasses.replace(
                    diag[:],
                    ap=[[TPC * E * KB, 128], [E * KB, 2], [KB, E], [2, KB // 2], [1, 2]],
                    offset=t0 * E * KB,
                )
                idrep_v = dataclasses.replace(
                    idrep_h[:],
                    ap=[[E * KB, 128], [0, 2], [KB, E], [2, KB // 2], [1, 2]],
                    offset=0,
                )
                g2_v = dataclasses.replace(
                    g2_sb[:],
                    ap=[[g2_pitch, 128], [E * 2, 2], [2, E], [0, KB // 2], [1, 2]],
                    offset=(c * TPC + t0) * E * 2,
                )
                eng.tensor_tensor(diag_v, idrep_v, g2_v, op=mybir.AluOpType.mult)

            yT_all = ytp.tile([128, E, TPC, 128], F16, tag="yT")
            for t in range(TPC):
                yT_ps = ps_yt.tile([128, NB, E, KB], F32, tag="yTps")
                for l in range(NB):
                    nc.tensor.matmul(
                        yT_ps[:, l, :, :],
                        x_h[l * KB : (l + 1) * KB, t, :],
                        diag[l * KB : (l + 1) * KB, t, :, :],
                        start=True,
                        stop=True,
                    )
                dst = dataclasses.replace(
                    yT_all[:],
                    ap=[[E * TPC * 128, 128], [KB, NB], [TPC * 128, E], [1, KB]],
                    offset=t * 128,
                )
                gather_engines[t](dst, yT_ps[:])
            return yT_all

        def emit_back(c, yT_all):
            row0 = c * CHUNK
            oT_ps = ps_ot.tile([128, CHUNK], F32, tag="oTps")
            for e in range(E):
                nc.tensor.matmul(
                    oT_ps[:],
                    alpha_h[:, e, :],
                    yT_all[:, e, :, :],
                    start=(e == 0),
                    stop=(e == E - 1),
                )

            oT_sb = op.tile([128, CHUNK], F16, tag="oT")
            nc.scalar.copy(oT_sb[:], oT_ps[:])
            nc.sync.dma_start(outT[:, row0 : row0 + CHUNK], oT_sb[:])

        pending = None
        for c in range(NCHUNK):
            front = emit_front(c)
            if pending is not None:
                emit_back(c - 1, pending)
            pending = front
        emit_back(NCHUNK - 1, pending)

    nc.compile()
    return nc


_NC_CACHE = None


def _make_idrep():
    idrep = np.zeros((128, E, KB), np.float16)
    for p in range(128):
        idrep[p, :, p % KB] = 1.0
    return idrep


def make_in_maps(x, gate_perc, alpha, beta):
    x = np.asarray(x, dtype=np.float32)
    gate_perc = np.asarray(gate_perc, dtype=np.float32)
    alpha = np.ascontiguousarray(np.asarray(alpha, dtype=np.float32))
    idrep = _make_idrep()
    in_maps = []
    for c in range(NCORES):
        sl = slice(c * BLOC, (c + 1) * BLOC)
        # xh[q, ch, t, i] = x[c*BLOC + 512ch + 128t + q, i] fp16
        xc = x[sl].astype(np.float16).reshape(NCHUNK, TPC, 128, I)
        xh = np.ascontiguousarray(xc.transpose(2, 0, 1, 3))
        # g2[q, ch, t, e, j] = gate[c*BLOC + 512ch + 128t + q, e], j duplicated
        gc = gate_perc[sl].astype(np.float16).reshape(NCHUNK, TPC, 128, E)
        gc = gc.transpose(2, 0, 1, 3)  # [q, ch, t, e]
        g2 = np.ascontiguousarray(
            np.broadcast_to(gc[..., None], (128, NCHUNK, TPC, E, 2))
        )
        in_maps.append(
            {
                "xh": xh,
                "g2": g2,
                "alpha": alpha,
                "idrep": idrep,
            }
        )
    return in_maps


def assemble(results, gate_perc, beta):
    # per-core outputs are [U, BLOC] f16; bias gate@beta added on host
    full_T = np.concatenate([results[c]["outT"] for c in range(NCORES)], axis=1)
    out = np.ascontiguousarray(full_T.T.astype(np.float32))
    out += np.asarray(gate_perc, dtype=np.float32) @ np.asarray(beta, dtype=np.float32)
    return out


def kernel(x, gate_perc, alpha, beta):
    global _NC_CACHE
    if _NC_CACHE is None:
        _NC_CACHE = _build()
    nc = _NC_CACHE

    in_maps = make_in_maps(x, gate_perc, alpha, beta)
    res = run_bass_kernel_spmd(nc, in_maps, list(range(NCORES))).results
    return assemble(res, gate_perc, beta)


if __name__ == "__main__":
    rng = np.random.default_rng(0)
    x = rng.standard_normal((B, I)).astype(np.float32)
    g = rng.random((B, E)).astype(np.float32)
    g /= g.sum(-1, keepdims=True)
    al = (rng.standard_normal((E, I, U)) * 0.05).astype(np.float32)
    be = (rng.standard_normal((E, U)) * 0.05).astype(np.float32)
    got = kernel(x, g, al, be)
    ref = np.einsum("bi,eio->beo", x, al, optimize=True)
    ref = np.einsum("beo,be->bo", ref, g) + g @ be
    err = np.abs(got - ref)
    print("max abs err", err.max(), "rel", err.max() / np.abs(ref).max())


# revision 10
# speedup vs baseline: 1.1077x; 1.0195x over previous
"""Trainium2 Bass kernel for nn_DenseExpert (soft-gated mixture of dense experts).

Math:  out[b,u] = sum_e gate[b,e] * (x[b,:] @ alpha[e]) [u] + (gate @ beta)[b,u]

Strategy (pure data parallel over batch, 8 cores). The back matmuls consume
yT_all[i, e, b] = gate[b, e] * x[b, i] from SBUF; that tensor is produced two
ways, balancing the PE against otherwise-idle DMA bandwidth:
  - experts 0..N_P-1 on the PE via the block-diag gated-transpose trick
    (diag builds on DVE/GPSIMD, front matmuls, ACT/DVE PSUM->SBUF gathers);
  - experts N_P..7 precomputed gated+transposed on the HOST (gxT) and DMA'd
    straight into the yT_all slice (no PE, no vector work).
Per 512-row chunk:
  1. x fp16 [128, chunk, tile, I] (host layout) loads as one contiguous DMA.
  2. diag[p, t, e, c] = gate[512c+128t+p, e] * [c == p%64], per-tile
     tensor_tensor ops: tile 0 on DVE (2x mode via host-duplicated gate
     pairs), tiles 1-3 on GPSIMD (all-SBUF, so allowed there).
  3. Front PE matmuls: for tile t, 64-row block l:
     yT[i, (e, c)] = x_block.T @ diag_block  (N = N_P*64).
  4. Gathers (PSUM fp32 -> SBUF fp16): tiles 0,2 on ACT, 1,3 on DVE.
  5. Back PE matmuls accumulate out.T[u,b] = sum_e alpha_e.T @ y_e.T into a
     double-buffered 1-bank PSUM tile; ACT copies it to SBUF as fp16.
  6. out.T fp16 DMA'd to DRAM [U, B]; host transposes/upcasts and adds the
     tiny bias gate @ beta while assembling the full result.
"""

import dataclasses
from contextlib import ExitStack

import numpy as np

import concourse.bacc as bacc
import concourse.tile as tile
import concourse.mybir as mybir
from concourse.bass_utils import run_bass_kernel_spmd

F32 = mybir.dt.float32
F16 = mybir.dt.float16

B, E, I, U = 65536, 8, 128, 128
NCORES = 8
BLOC = B // NCORES        # 8192 batch rows per core
CHUNK = 512               # batch rows per pipeline chunk
NCHUNK = BLOC // CHUNK    # 16
TPC = CHUNK // 128        # 128-row tiles per chunk
KB = 64                   # contraction block for the diag trick
NB = 128 // KB            # diag blocks per 128-row tile
N_P = 8                   # experts computed on the PE
N_D = E - N_P             # experts pre-gated on the host, DMA'd in


def _build():
    nc = bacc.Bacc("TRN2", target_bir_lowering=False, debug=False)

    # xh[q, c, t, i] = x[512c + 128t + q, i] as fp16 (host-prepared)
    xh = nc.dram_tensor("xh", [128, NCHUNK, TPC, I], F16, kind="ExternalInput").ap()
    # g2[q, c, t, e, j] = gate[512c + 128t + q, e] duplicated over j in {0,1}
    g2 = nc.dram_tensor("g2", [128, NCHUNK, TPC, N_P, 2], F16, kind="ExternalInput").ap()
    alpha = nc.dram_tensor("alpha", [E, I, U], F32, kind="ExternalInput").ap()
    idrep = nc.dram_tensor("idrep", [128, N_P, KB], F16, kind="ExternalInput").ap()
    # gxt[i, c, ed, t, q] = gate[512c+128t+q, N_P+ed] * x[512c+128t+q, i] fp16
    gxt = None
    if N_D:
        gxt = nc.dram_tensor(
            "gxt", [128, NCHUNK, N_D, TPC, 128], F16, kind="ExternalInput"
        ).ap()
    # output stays feature-major fp16 on HW; host transposes/upcasts
    outT = nc.dram_tensor("outT", [U, BLOC], F16, kind="ExternalOutput").ap()

    with tile.TileContext(nc) as tc, ExitStack() as ctx:
        const = ctx.enter_context(tc.tile_pool(name="const", bufs=1))
        xp = ctx.enter_context(tc.tile_pool(name="xp", bufs=4))
        dgp = ctx.enter_context(tc.tile_pool(name="dgp", bufs=3))
        ytp = ctx.enter_context(tc.tile_pool(name="ytp", bufs=3))
        op = ctx.enter_context(tc.tile_pool(name="op", bufs=3))
        # PSUM: 3 yT tiles + 2 output buffers
        ps_yt = ctx.enter_context(tc.tile_pool(name="ps_yt", bufs=3, space="PSUM"))
        ps_ot = ctx.enter_context(tc.tile_pool(name="ps_ot", bufs=2, space="PSUM"))

        # consts needed by the first front matmuls go first on the DMA queue
        g2_sb = const.tile([128, NCHUNK, TPC, N_P, 2], F16, tag="g2")
        nc.sync.dma_start(g2_sb[:], g2)
        g2_pitch = NCHUNK * TPC * N_P * 2
        idrep_h = const.tile([128, N_P, KB], F16, tag="idreph")
        nc.sync.dma_start(idrep_h[:], idrep)

        alpha_sb = const.tile([128, E, U], F32, tag="alpha")
        alpha_h = const.tile([128, E, U], F16, tag="alphah")

        def emit_alpha():
            # alpha is only needed by the first back matmuls (one chunk in);
            # emitted after front(0) so it doesn't block the first diag/matmul
            nc.sync.dma_start(alpha_sb[:], alpha.rearrange("e i u -> i e u"))
            nc.scalar.copy(alpha_h[:], alpha_sb[:])

        gather_engines = [
            nc.scalar.copy,
            nc.vector.tensor_copy,
            nc.scalar.copy,
            nc.vector.tensor_copy,
        ]

        def emit_front(c):
            x_h = xp.tile([128, TPC, I], F16, tag="xh")
            nc.sync.dma_start(x_h[:], xh[:, c])

            yT_all = ytp.tile([128, E, TPC, 128], F16, tag="yT")
            if N_D:
                # host-pregated experts: straight DMA into the yT slice
                nc.sync.dma_start(
                    dataclasses.replace(
                        yT_all[:],
                        ap=[[E * TPC * 128, 128], [1, N_D * TPC * 128]],
                        offset=N_P * TPC * 128,
                    ),
                    gxt[:, c],
                )

            # diag matrices: tile 0 on DVE (2x stride-1 pairs), 1-3 on GPSIMD
            diag = dgp.tile([128, TPC, N_P, KB], F16, tag="diag")
            for t in range(TPC):
                eng = nc.vector if t < 2 else nc.gpsimd
                diag_v = dataclasses.replace(
                    diag[:],
                    ap=[[TPC * N_P * KB, 128], [KB, N_P], [2, KB // 2], [1, 2]],
                    offset=t * N_P * KB,
                )
                idrep_v = dataclasses.replace(
                    idrep_h[:],
                    ap=[[N_P * KB, 128], [KB, N_P], [2, KB // 2], [1, 2]],
                    offset=0,
                )
                g2_v = dataclasses.replace(
                    g2_sb[:],
                    ap=[[g2_pitch, 128], [2, N_P], [0, KB // 2], [1, 2]],
                    offset=(c * TPC + t) * N_P * 2,
                )
                eng.tensor_tensor(diag_v, idrep_v, g2_v, op=mybir.AluOpType.mult)

            for t in range(TPC):
                yT_ps = ps_yt.tile([128, NB, N_P, KB], F32, tag="yTps")
                for l in range(NB):
                    nc.tensor.matmul(
                        yT_ps[:, l, :, :],
                        x_h[l * KB : (l + 1) * KB, t, :],
                        diag[l * KB : (l + 1) * KB, t, :, :],
                        start=True,
                        stop=True,
                    )
                dst = dataclasses.replace(
                    yT_all[:],
                    ap=[[E * TPC * 128, 128], [KB, NB], [TPC * 128, N_P], [1, KB]],
                    offset=t * 128,
                )
                gather_engines[t](dst, yT_ps[:])
            return yT_all

        def emit_back(c, yT_all):
            row0 = c * CHUNK
            oT_ps = ps_ot.tile([128, CHUNK], F32, tag="oTps")
            for e in range(E):
                nc.tensor.matmul(
                    oT_ps[:],
                    alpha_h[:, e, :],
                    yT_all[:, e, :, :],
                    start=(e == 0),
                    stop=(e == E - 1),
                )

            oT_sb = op.tile([128, CHUNK], F16, tag="oT")
            nc.scalar.copy(oT_sb[:], oT_ps[:])
            nc.sync.dma_start(outT[:, row0 : row0 + CHUNK], oT_sb[:])

        pending = None
        for c in range(NCHUNK):
            front = emit_front(c)
            if c == 0:
                emit_alpha()
            if pending is not None:
                emit_back(c - 1, pending)
            pending = front
        emit_back(NCHUNK - 1, pending)

    nc.compile()
    return nc


_NC_CACHE = None


def _make_idrep():
    idrep = np.zeros((128, N_P, KB), np.float16)
    for p in range(128):
        idrep[p, :, p % KB] = 1.0
    return idrep


def make_in_maps(x, gate_perc, alpha, beta):
    x = np.asarray(x, dtype=np.float32)
    gate_perc = np.asarray(gate_perc, dtype=np.float32)
    alpha = np.ascontiguousarray(np.asarray(alpha, dtype=np.float32))
    idrep = _make_idrep()
    in_maps = []
    for c in range(NCORES):
        sl = slice(c * BLOC, (c + 1) * BLOC)
        xc = x[sl].reshape(NCHUNK, TPC, 128, I)
        gc = gate_perc[sl].reshape(NCHUNK, TPC, 128, E)
        # xh[q, ch, t, i] fp16
        xh = np.ascontiguousarray(
            xc.transpose(2, 0, 1, 3).astype(np.float16)
        )
        # g2[q, ch, t, e, j] fp16, j duplicated, PE experts only
        g2p = gc.transpose(2, 0, 1, 3)[..., :N_P].astype(np.float16)
        g2 = np.ascontiguousarray(
            np.broadcast_to(g2p[..., None], (128, NCHUNK, TPC, N_P, 2))
        )
        m = {"xh": xh, "g2": g2, "alpha": alpha, "idrep": idrep}
        if N_D:
            # gxt[i, ch, ed, t, q] = gate[.., N_P+ed] * x[.., i] fp16
            gx = xc[..., None, :] * gc[..., N_P:, None]  # [ch,t,q,ed,i] f32
            m["gxt"] = np.ascontiguousarray(
                gx.transpose(4, 0, 3, 1, 2).astype(np.float16)
            )
        in_maps.append(m)
    return in_maps


def assemble(results, gate_perc, beta):
    # per-core outputs are [U, BLOC] f16; bias gate@beta added on host
    full_T = np.concatenate([results[c]["outT"] for c in range(NCORES)], axis=1)
    out = np.ascontiguousarray(full_T.T.astype(np.float32))
    out += np.asarray(gate_perc, dtype=np.float32) @ np.asarray(beta, dtype=np.float32)
    return out


def kernel(x, gate_perc, alpha, beta):
    global _NC_CACHE
    if _NC_CACHE is None:
        _NC_CACHE = _build()
    nc = _NC_CACHE

    in_maps = make_in_maps(x, gate_perc, alpha, beta)
    res = run_bass_kernel_spmd(nc, in_maps, list(range(NCORES))).results
    return assemble(res, gate_perc, beta)


if __name__ == "__main__":
    rng = np.random.default_rng(0)
    x = rng.standard_normal((B, I)).astype(np.float32)
    g = rng.random((B, E)).astype(np.float32)
    g /= g.sum(-1, keepdims=True)
    al = (rng.standard_normal((E, I, U)) * 0.05).astype(np.float32)
    be = (rng.standard_normal((E, U)) * 0.05).astype(np.float32)
    got = kernel(x, g, al, be)
    ref = np.einsum("bi,eio->beo", x, al, optimize=True)
    ref = np.einsum("beo,be->bo", ref, g) + g @ be
    err = np.abs(got - ref)
    print("max abs err", err.max(), "rel", err.max() / np.abs(ref).max())
